# revision 7
# baseline (speedup 1.0000x reference)
"""CTC loss (nn_CTCLoss) on 8 Trainium2 NeuronCores — indirect-gather +
bidirectional scan design.

kernel(predicts [256,160,6625] f32 log-probs, labels [256,25] i32,
       label_lengths [256]) -> scalar f32 mean CTC loss.

Sharding: batch 256 -> 8 cores x 32.  Each core receives its predicts shard
host-transposed to class-major layout predT [32, 6626, 160] (class 6625 is a
-1e30 sentinel column), so each (batch, class) time-series is one contiguous
640B row.  The device gathers ONLY the rows it needs (25 label slots + blank
per batch = 832 rows ~ 0.5 MB instead of streaming the full 135 MB shard):

  1. 7 indirect DMAs (gpsimd.indirect_dma_start, one int32 row index per
     partition) pull 128 rows each; partition 32q+b of call u holds label
     slot j=4u+q of batch b.
  2. 4 strided SBUF->SBUF DMAs repack into G [32, (j*160+t)] batch-major
     (+ slot j=24 and the blank row separately).
  3. ACT exp: P = exp(G + BIAS); sentinel rows -> 0, exactly killing label
     slots j >= len(b).
  4. DVE bidirectional scan in probability space, 3 ops/step, two
     INDEPENDENT dependency chains interleaved (hides the ~90ns dependent-op
     stall; ~109ns/op instead of ~200ns):
       forward (t=1..80), state u[i] = alpha[2i]+alpha[2i-1]-style even/odd
       split:  v = u[0:25]+O;  O' = v*pl_t;  u' = u*pb_t + shift(O')
       backward (t=159..81), beta even/odd split:
         go = Bo*pl_t;  Be' = Be*pb_t + [go,0];  Bo' = go + Be'[1:]
     (pb_t is a per-partition scalar -> scalar_tensor_tensor fuses mult+add.)
     Each chain renormalizes by its max every 32 steps and at its end (log
     accumulated).
     Neither chain reaches the regime where dead-path mass can swamp the
     renorm max, so no viability masking is needed at all.
  5. merge at t=80: ll = sum_s alpha_80[s]*beta_80[s];
     loss_b = BIAS*T - (ln(ll) + accF + accB).

Valid when no adjacent labels repeat (all skip transitions allowed); samples
with adjacent repeated labels (~1 in 256 random draws) are recomputed exactly
on the host in float64 and substituted before the mean.
"""

import numpy as np

import concourse.bass as bass
import concourse.mybir as mybir
import concourse.tile as tile
from concourse import bacc
from concourse.bass_utils import run_bass_kernel_spmd

F32 = mybir.dt.float32
BF16 = mybir.dt.bfloat16
I32 = mybir.dt.int32

N_CORES = 8
B_FULL = 256
B = 32          # batch per core
T = 160
T0 = 80         # meet-in-the-middle point; fwd covers t<=80, bwd t>=81
C = 6625
CP = C + 1      # + sentinel class (-1e30)
NROWS = B * CP
S = 25
W = 52          # state width: even cols 0..25, guard col 26, odd at 27..51
RENORM = 32
BIAS = 8.8
NCALL = 7       # gather calls: 6x4 label slots + [j24, blank, -, -]


def _prep_core_inputs(pred, labels, lens):
    """One core's shard -> device input dict."""
    lens = lens.astype(np.int64)
    labels = labels.astype(np.int64)

    predT = np.empty((B, CP, T), dtype=np.float32)
    predT[:, :C, :] = pred.transpose(0, 2, 1)
    predT[:, C, :] = -1e30

    # row index per (batch, slot): slot j<25 -> label j (sentinel if j>=len),
    # call 6: q=0 -> slot 24, q=1 -> blank row, q=2,3 -> sentinel (unused).
    cls = np.where(np.arange(S)[None, :] < lens[:, None], labels, C)  # [B,25]
    idx128 = np.full((128, NCALL), C, dtype=np.int64)  # default sentinel
    for q in range(4):
        for u in range(6):
            j = 4 * u + q
            if j < S:
                idx128[32 * q : 32 * q + 32, u] = cls[:, j]
    idx128[0:32, 6] = cls[:, 24]
    idx128[32:64, 6] = 0  # blank row
    b_off = np.tile(np.arange(B) * CP, 4).reshape(128)
    idx128 = (idx128 + b_off[:, None]).astype(np.int32)

    mfin = np.zeros((B, W), dtype=np.float32)
    bi = np.arange(B)
    mfin[bi, lens] = 1.0          # beta init: even position s=2*len
    mfin[bi, 26 + lens] = 1.0     # beta init: odd position s=2*len-1

    return {
        "predT": np.ascontiguousarray(predT.reshape(NROWS, T)),
        "idx": idx128,
        "mfin": mfin,
    }


def _gather(nc, predT, sb_idx, g4, g, pbl, pw, pbw, sb_bias):
    """Gather all needed rows, repack to batch-major, exp to bf16."""
    for u in range(NCALL):
        nc.gpsimd.indirect_dma_start(
            out=g4[:, T * u : T * (u + 1)],
            out_offset=None,
            in_=predT[:, :],
            in_offset=bass.IndirectOffsetOnAxis(ap=sb_idx[:, u : u + 1], axis=0),
        )
    gv = g[:, :].rearrange("p (u q tt) -> p u q tt", q=4, tt=T)
    for q in range(4):
        nc.sync.dma_start(
            gv[:, 0:6, q, :],
            g4[32 * q : 32 * q + 32, 0 : 6 * T].rearrange(
                "p (u tt) -> p u tt", tt=T
            ),
        )
    nc.sync.dma_start(g[:, 160 * 24 : 160 * 25], g4[0:32, 6 * T : 7 * T])
    nc.sync.dma_start(pbl[:, :], g4[32:64, 6 * T : 7 * T])
    nc.scalar.activation(
        pw[:, :], g[:, 0 : 160 * S],
        mybir.ActivationFunctionType.Exp, bias=sb_bias[:, :], scale=1.0,
    )
    nc.scalar.activation(
        pbw[:, :], pbl[:, :],
        mybir.ActivationFunctionType.Exp, bias=sb_bias[:, :], scale=1.0,
    )


def _renorm(nc, st, red, rec, lred, acc):
    nc.vector.tensor_reduce(
        red[:, :], st[:, :], axis=mybir.AxisListType.X, op=mybir.AluOpType.max
    )
    nc.vector.reciprocal(rec[:, :], red[:, :])
    nc.vector.tensor_scalar_mul(st[:, :], st[:, :], rec[:, :])
    nc.scalar.activation(lred[:, :], red[:, :], mybir.ActivationFunctionType.Ln)
    nc.vector.tensor_tensor(
        acc[:, :], acc[:, :], lred[:, :], op=mybir.AluOpType.add
    )


def _pipeline(nc, predT, loss_ap, sb_idx, sb_mfin, sb_bias, g4, g, pbl,
              pw, pbw, stA, stB, tmpA, tmpB, redF, recF, accF, lredF, redB,
              recB, accB, lredB, loss_sb, loss_acc):
    _gather(nc, predT, sb_idx, g4, g, pbl, pw, pbw, sb_bias)

    gj = pw[:, :].rearrange("p (j tt) -> p j tt", tt=T)  # [32,25,160] bf16

    # init forward state: u = [pb0, pl0[0], 0...], O = [pl0[0], 0...]
    nc.vector.memset(stA[:, :], 0.0)
    nc.vector.memset(accF[:, :], 0.0)
    nc.vector.tensor_copy(stA[:, 0:1], pbw[:, 0:1])
    nc.vector.tensor_copy(stA[:, 1:2], pw[:, 0:1])
    nc.vector.tensor_copy(stA[:, 27:28], pw[:, 0:1])
    # init backward state: Be[len]=1, Bo[len-1]=1
    nc.vector.tensor_copy(stB[:, :], sb_mfin[:, :])
    nc.vector.memset(accB[:, :], 0.0)
    nc.vector.memset(tmpB[:, :], 0.0)  # col 25 stays 0 (go padding)

    for r in range(T0):
        tf = 1 + r          # forward t: 1..80
        tb = 159 - r        # backward t: 159..80 (skip last at 80)
        # forward step
        nc.vector.tensor_tensor(
            tmpA[:, :], stA[:, 0:25], stA[:, 27:52], op=mybir.AluOpType.add
        )
        nc.vector.tensor_tensor(
            stA[:, 27:52], tmpA[:, :], gj[:, :, tf], op=mybir.AluOpType.mult
        )
        nc.vector.scalar_tensor_tensor(
            stA[:, 0:26], stA[:, 0:26], pbw[:, tf : tf + 1], stA[:, 26:52],
            op0=mybir.AluOpType.mult, op1=mybir.AluOpType.add,
        )
        # backward step (79 steps: t=159..81)
        if tb >= 81:
            nc.vector.tensor_tensor(
                tmpB[:, 0:25], stB[:, 27:52], gj[:, :, tb],
                op=mybir.AluOpType.mult,
            )
            nc.vector.scalar_tensor_tensor(
                stB[:, 0:26], stB[:, 0:26], pbw[:, tb : tb + 1], tmpB[:, 0:26],
                op0=mybir.AluOpType.mult, op1=mybir.AluOpType.add,
            )
            nc.vector.tensor_tensor(
                stB[:, 27:52], tmpB[:, 0:25], stB[:, 1:26],
                op=mybir.AluOpType.add,
            )
        if (r + 1) % RENORM == 0 or r == T0 - 1:
            _renorm(nc, stA, redF, recF, lredF, accF)
            _renorm(nc, stB, redB, recB, lredB, accB)

    # merge at t=80: E = u - shift(O); ll = sum(E*Be) + sum(O*Bo).
    # The dot product can be far below the ACT Ln table's ~1e-20 floor, so
    # rescale the product tile by its max first and recover ln(max) through
    # sqrt (2*Ln(Sqrt(m)) keeps the table input in range).
    nc.vector.tensor_tensor(
        stA[:, 0:26], stA[:, 0:26], stA[:, 26:52], op=mybir.AluOpType.subtract
    )
    nc.vector.tensor_tensor(
        stB[:, 0:26], stB[:, 0:26], stA[:, 0:26], op=mybir.AluOpType.mult
    )
    nc.vector.tensor_tensor(
        stB[:, 27:52], stB[:, 27:52], stA[:, 27:52], op=mybir.AluOpType.mult
    )
    nc.vector.tensor_reduce(
        redF[:, :], stB[:, :], axis=mybir.AxisListType.X, op=mybir.AluOpType.max
    )
    nc.vector.reciprocal(recF[:, :], redF[:, :])
    nc.vector.tensor_scalar_mul(stB[:, :], stB[:, :], recF[:, :])
    nc.vector.tensor_reduce(
        redB[:, :], stB[:, :], axis=mybir.AxisListType.X, op=mybir.AluOpType.add
    )
    nc.scalar.activation(
        lredF[:, :], redB[:, :], mybir.ActivationFunctionType.Ln
    )
    nc.scalar.activation(
        recB[:, :], redF[:, :], mybir.ActivationFunctionType.Sqrt
    )
    nc.scalar.activation(
        lredB[:, :], recB[:, :], mybir.ActivationFunctionType.Ln
    )
    nc.vector.tensor_scalar(
        lredB[:, :], lredB[:, :], 2.0, 0.0,
        op0=mybir.AluOpType.mult, op1=mybir.AluOpType.add,
    )
    nc.vector.tensor_tensor(
        lredF[:, :], lredF[:, :], lredB[:, :], op=mybir.AluOpType.add
    )
    nc.vector.tensor_tensor(
        lredF[:, :], lredF[:, :], accF[:, :], op=mybir.AluOpType.add
    )
    nc.vector.tensor_tensor(
        lredF[:, :], lredF[:, :], accB[:, :], op=mybir.AluOpType.add
    )
    nc.vector.tensor_scalar(
        loss_sb[:, :], lredF[:, :], -1.0, BIAS * T,
        op0=mybir.AluOpType.mult, op1=mybir.AluOpType.add,
    )
    # accumulate into the live output (keeps every repeat's work live for
    # the K-repeat timing NEFFs; with repeats=1 loss_acc == loss_sb)
    nc.vector.tensor_tensor(
        loss_acc[:, :], loss_acc[:, :], loss_sb[:, :], op=mybir.AluOpType.add
    )


def _emit(tc, predT, idx_ap, mfin_ap, loss_ap, repeats=1):
    nc = tc.nc
    with tc.tile_pool(name="state", bufs=1) as pool:
        sb_idx = pool.tile([128, NCALL], I32, name="sb_idx")
        nc.sync.dma_start(sb_idx[:, :], idx_ap[:, :])
        sb_mfin = pool.tile([B, W], F32, name="sb_mfin")
        nc.sync.dma_start(sb_mfin[:, :], mfin_ap[:, :])
        sb_bias = pool.tile([B, 1], F32, name="sb_bias")
        nc.vector.memset(sb_bias[:, :], BIAS)

        g4 = pool.tile([128, NCALL * T], F32, name="g4")
        g = pool.tile([B, 28 * T], F32, name="g")  # 25 slots + 3 pad
        pbl = pool.tile([B, T], F32, name="pbl")
        pw = pool.tile([B, S * T], BF16, name="pw")
        pbw = pool.tile([B, T], BF16, name="pbw")
        stA = pool.tile([B, W], BF16, name="stA")
        stB = pool.tile([B, W], BF16, name="stB")
        tmpA = pool.tile([B, S], BF16, name="tmpA")
        tmpB = pool.tile([B, 26], BF16, name="tmpB")
        redF = pool.tile([B, 1], F32, name="redF")
        recF = pool.tile([B, 1], F32, name="recF")
        accF = pool.tile([B, 1], F32, name="accF")
        lredF = pool.tile([B, 1], F32, name="lredF")
        redB = pool.tile([B, 1], F32, name="redB")
        recB = pool.tile([B, 1], F32, name="recB")
        accB = pool.tile([B, 1], F32, name="accB")
        lredB = pool.tile([B, 1], F32, name="lredB")
        loss_sb = pool.tile([B, 1], F32, name="loss_sb")
        loss_acc = pool.tile([B, 1], F32, name="loss_acc")

        nc.vector.memset(loss_acc[:, :], 0.0)
        for _ in range(repeats):
            _pipeline(nc, predT, loss_ap, sb_idx, sb_mfin, sb_bias, g4,
                      g, pbl, pw, pbw, stA, stB, tmpA, tmpB, redF, recF, accF,
                      lredF, redB, recB, accB, lredB, loss_sb, loss_acc)
        if repeats > 1:
            # report the average so the K-NEFF output matches the K=1 output
            nc.vector.tensor_scalar(
                loss_acc[:, :], loss_acc[:, :], 1.0 / repeats, 0.0,
                op0=mybir.AluOpType.mult, op1=mybir.AluOpType.add,
            )
        nc.sync.dma_start(loss_ap[:, :], loss_acc[:, :])


_CACHED_NC = None


def build_nc(repeats=1):
    global _CACHED_NC
    if _CACHED_NC is not None and repeats == 1:
        return _CACHED_NC
    nc = bacc.Bacc("TRN2", target_bir_lowering=False, debug=False,
                   num_devices=N_CORES)
    predT = nc.dram_tensor("predT", [NROWS, T], F32, kind="ExternalInput").ap()
    idx = nc.dram_tensor("idx", [128, NCALL], I32, kind="ExternalInput").ap()
    mfin = nc.dram_tensor("mfin", [B, W], F32, kind="ExternalInput").ap()
    loss = nc.dram_tensor("loss", [B, 1], F32, kind="ExternalOutput").ap()
    with tile.TileContext(nc) as tc:
        _emit(tc, predT, idx, mfin, loss, repeats=repeats)
    nc.compile()
    if repeats == 1:
        _CACHED_NC = nc
    return nc


def make_in_maps(predicts, labels, label_lengths):
    predicts = np.asarray(predicts, dtype=np.float32)
    labels = np.asarray(labels)
    lens = np.asarray(label_lengths).astype(np.int64)
    in_maps = []
    for c in range(N_CORES):
        sl = slice(c * B, (c + 1) * B)
        in_maps.append(_prep_core_inputs(predicts[sl], labels[sl], lens[sl]))
    return in_maps


def _ref_ctc_loss_one(lp, labels, ln):
    """Exact single-sample CTC loss (float64 log space) for repeat samples."""
    L = 2 * S + 1
    ext = np.zeros(L, np.int64)
    ext[1::2] = labels
    lp_ext = lp[:, ext]
    prev2 = np.full(L, -1, np.int64)
    prev2[2:] = ext[:-2]
    allow = (ext != 0) & (ext != prev2)
    NEG = -1e30
    alpha = np.full(L, NEG)
    alpha[0] = lp_ext[0, 0]
    alpha[1] = lp_ext[0, 1]
    for t in range(1, T):
        a1 = np.concatenate([[NEG], alpha[:-1]])
        a2 = np.concatenate([[NEG, NEG], alpha[:-2]])
        a2 = np.where(allow, a2, NEG)
        m = np.maximum(alpha, np.maximum(a1, a2))
        alpha = m + np.log(
            np.exp(alpha - m) + np.exp(a1 - m) + np.exp(a2 - m)
        ) + lp_ext[t]
    i = 2 * ln
    m = max(alpha[i], alpha[i - 1])
    return -(m + np.log(np.exp(alpha[i] - m) + np.exp(alpha[i - 1] - m)))


def kernel(predicts, labels, label_lengths):
    predicts = np.asarray(predicts, dtype=np.float32)
    labels = np.asarray(labels)
    lens = np.asarray(label_lengths).astype(np.int64)
    nc = build_nc()
    in_maps = make_in_maps(predicts, labels, lens)
    res = run_bass_kernel_spmd(nc, in_maps, core_ids=list(range(N_CORES)))
    losses = np.concatenate(
        [res.results[c]["loss"].reshape(B) for c in range(N_CORES)]
    )
    # exact host recomputation for samples where a skip transition is
    # forbidden (adjacent repeated labels) — the fast scan allows all skips
    rep = (labels[:, 1:] == labels[:, :-1]) & (
        np.arange(1, S)[None, :] < lens[:, None]
    )
    for b in np.where(rep.any(axis=1))[0]:
        losses[b] = _ref_ctc_loss_one(
            predicts[b].astype(np.float64), labels[b].astype(np.int64), lens[b]
        )
    return np.float32(losses.mean())
